# revision 5
# baseline (speedup 1.0000x reference)
"""Trainium2 Bass kernel for nn_ChatDecoder: greedy LSTM decoder, 32 steps.

v2 over baseline: the fp16 two-term dense runs in TWO passes instead of
three.  Pass A packs [A1s | A2] as a [128,128] lhsT (M=128) so A1s@W1 and
A2@W1 share one PE column stream (same cycles as one M=64 pass); pass B
accumulates A1@W2 into PSUM rows 0-63.  logits = (psum_low + psum_high)/2^11,
combined via scalar-engine copy (psum_high/SC -> SBUF; DVE may read only one
PSUM operand per instruction) + one vector scalar_tensor_tensor.  Same trick
for the h @ W_hh gate matmuls.  Dense PE time drops ~42.7us -> ~26.7us/step.

Everything else follows the baseline: vocab-sharded dense + unit-sharded
LSTM on 8 cores, h exchanged per step via AllGather (one packed [128,192]
f16 slot per core: [A1s|A2|A1]), argmax via per-shard top-1 + tiny AllGather
+ first-occurrence combine, host-precomputed f64 videmb = emb@W_ih + b
gathered by indirect DMA, tanh-only gates.  Argmax margins go down to
1.4e-6 so the whole argmax path stays fp32-or-better.

Output per core: [32, 64, 4000]; host concatenates shards + transposes.
"""

import sys
from contextlib import ExitStack

import numpy as np

for _p in ("/opt/trn_rl_repo",):
    if _p not in sys.path:
        sys.path.insert(0, _p)

import concourse.bass as bass
import concourse.tile as tile
from concourse import bacc, mybir
from concourse.bass_utils import run_bass_kernel_spmd

F32 = mybir.dt.float32
F16 = mybir.dt.float16
I32 = mybir.dt.int32
U32 = mybir.dt.uint32
TANH = mybir.ActivationFunctionType.Tanh
COPY = mybir.ActivationFunctionType.Copy
OP = mybir.AluOpType
X = mybir.AxisListType.X

V, E, U, B, T_FULL = 32000, 512, 1024, 64, 32
NC = 8
VS = V // NC          # 4000 vocab shard
NT = 500              # dense moving tile (legacy name; see TSZ)
NTILES = 8
TSZ = [512] * 7 + [416]           # uneven: small final tile shortens the tail
TOFF = [sum(TSZ[:i]) for i in range(NTILES)]
KD = U // 128         # 8 K-chunks
GO = 1
RG = [list(range(NC))]
SC = 2048.0           # 2^11 split scale
SLOT = 3 * 64         # [A1s | A2 | A1] per source slot, fp16


def build_program(T: int = T_FULL, has_bd: bool = False):
    nc = bacc.Bacc(
        "TRN2", target_bir_lowering=False, debug=False, num_devices=NC
    )

    def inp(name, shape, dtype=F32):
        return nc.dram_tensor(name, list(shape), dtype, kind="ExternalInput")

    hall0_d = inp("hall0", (128, NC * SLOT), F16)   # pre-cell h0 splits
    c0_d = inp("c0", (64, 128))
    videmb_d = inp("videmb", (V, 512))
    zx0_d = inp("zx0", (64, 512))
    whh1_d = inp("whh1", (128, KD * 512), F16)
    whh2_d = inp("whh2", (128, KD * 512), F16)
    wd1_d = inp("wd1", (128, KD * VS), F16)
    wd2_d = inp("wd2", (128, KD * VS), F16)
    offs_d = inp("offs8", (64, NTILES))
    id_d = inp("ident", (64, 64))
    if has_bd:
        bd_d = inp("bd", (64, VS))
    out_d = nc.dram_tensor("out", [T, B, VS], F32, kind="ExternalOutput")

    with tile.TileContext(nc) as tc, ExitStack() as ctx:
        const = ctx.enter_context(tc.tile_pool(name="const", bufs=1))
        hpool = ctx.enter_context(tc.tile_pool(name="hpool", bufs=2))
        gates = ctx.enter_context(tc.tile_pool(name="gates", bufs=2))
        cpool = ctx.enter_context(tc.tile_pool(name="cpool", bufs=2))
        zxpool = ctx.enter_context(tc.tile_pool(name="zxpool", bufs=2))
        lpool = ctx.enter_context(tc.tile_pool(name="lpool", bufs=1))
        ampool = ctx.enter_context(tc.tile_pool(name="ampool", bufs=2))
        dram = ctx.enter_context(tc.tile_pool(name="dram", bufs=2, space="DRAM"))
        zpsum = ctx.enter_context(tc.tile_pool(name="zpsum", bufs=2, space="PSUM"))
        dpsum = ctx.enter_context(tc.tile_pool(name="dpsum", bufs=4, space="PSUM"))
        tpsum = ctx.enter_context(tc.tile_pool(name="tpsum", bufs=2, space="PSUM"))

        whh1 = const.tile([128, KD * 512], F16)
        nc.sync.dma_start(whh1[:], whh1_d[:])
        whh2 = const.tile([128, KD * 512], F16)
        nc.sync.dma_start(whh2[:], whh2_d[:])
        wd1 = const.tile([128, KD * VS], F16)
        nc.sync.dma_start(wd1[:], wd1_d[:])
        wd2 = const.tile([128, KD * VS], F16)
        nc.sync.dma_start(wd2[:], wd2_d[:])
        offs = const.tile([64, NTILES], F32)
        nc.sync.dma_start(offs[:], offs_d[:])
        idn = const.tile([64, 64], F32)
        nc.sync.dma_start(idn[:], id_d[:])
        if has_bd:
            bd = const.tile([64, VS], F32)
            nc.sync.dma_start(bd[:], bd_d[:])
        hall0 = const.tile([128, NC * SLOT], F16, name="hall0")
        nc.sync.dma_start(hall0[:], hall0_d[:])

        c_cur = cpool.tile([64, 128], F32, name="c_sb")
        nc.sync.dma_start(c_cur[:], c0_d[:])
        zx_cur = zxpool.tile([64, 512], F32, name="zx_sb")
        nc.sync.dma_start(zx_cur[:], zx0_d[:])

        # PE must observe each DMA-loaded tensor it reads via one dummy
        # matmul each (self-loading matmuls tolerate only one sync wait).
        wps = tpsum.tile([128, 64], F32, name="tph")
        for src in (whh1, whh2, wd1, wd2, idn, hall0):
            nc.tensor.matmul(
                wps[0:1, 0:1], lhsT=src[0:32, 0:1], rhs=src[0:32, 0:1],
                start=True, stop=True, skip_group_check=True,
            )

        def emit_z_h(zps, hv):
            # SC * z_h in two passes: rows 0-63 get A1s@Whh1 + A1@Whh2,
            # rows 64-127 get A2@Whh1.
            for k in range(KD):
                nc.tensor.matmul(
                    zps[:],
                    lhsT=hv[:, SLOT * k : SLOT * k + 128],
                    rhs=whh1[:, 512 * k : 512 * (k + 1)],
                    start=(k == 0),
                    stop=False,
                )
            for k in range(KD):
                nc.tensor.matmul(
                    zps[0:64, :],
                    lhsT=hv[:, SLOT * k + 128 : SLOT * (k + 1)],
                    rhs=whh2[:, 512 * k : 512 * (k + 1)],
                    start=False,
                    stop=(k == KD - 1),
                    skip_group_check=True,
                )

        zps_cur = zpsum.tile([128, 512], F32, name="zps")
        emit_z_h(zps_cur, hall0)
        hv = hall0

        for t in range(T):
            zps = zps_cur
            zx = zx_cur

            # ---- z = (zps_low + zps_high)/SC + zx ----
            # zhi/zsum depend only on the h-matmuls, so they fill the argmax
            # wait; only the final +zx add sits after the gather.
            zhi = gates.tile([64, 512], F32, name="zhi")
            nc.scalar.activation(zhi[:], zps[64:128, :], COPY, scale=1.0 / SC)
            zsum = gates.tile([64, 512], F32, name="zsum")
            nc.vector.scalar_tensor_tensor(
                zsum[:], zps[0:64, :], 1.0 / SC, zhi[:], OP.mult, OP.add
            )
            z_sb = gates.tile([64, 512], F32, name="z_sb")
            nc.vector.tensor_add(z_sb[:], zsum[:], zx[:])

            # ---- LSTM cell (gate cols i,f,o,g; g carries host-side x2) ----
            tact = gates.tile([64, 512], F32, name="tact")
            nc.scalar.activation(tact[:], z_sb[:], TANH, scale=0.5)
            sig3 = gates.tile([64, 384], F32, name="sig3")
            nc.vector.tensor_scalar(sig3[:], tact[:, 0:384], 0.5, 0.5, OP.mult, OP.add)
            si, sf, so = sig3[:, 0:128], sig3[:, 128:256], sig3[:, 256:384]
            tg = tact[:, 384:512]
            q1 = gates.tile([64, 128], F32, name="q1")
            nc.vector.tensor_mul(q1[:], sf, c_cur[:])
            q2 = gates.tile([64, 128], F32, name="q2")
            nc.vector.tensor_mul(q2[:], si, tg)
            c_new = cpool.tile([64, 128], F32, name="c_sb")
            nc.vector.tensor_add(c_new[:], q1[:], q2[:])
            c_cur = c_new
            tcn = gates.tile([64, 128], F32, name="tcn")
            nc.scalar.activation(tcn[:], c_new[:], TANH)
            hnew = gates.tile([64, 128], F32, name="hnew")
            nc.vector.tensor_mul(hnew[:], so, tcn[:])

            # ---- transpose h slice, fp16 split [A1s|A2|A1], AllGather ----
            tph = tpsum.tile([128, 64], F32, name="tph")
            nc.tensor.transpose(tph[:], hnew[:], idn[:])
            spl = gates.tile([128, SLOT], F16, name="spl")
            nc.vector.tensor_copy(spl[:, 128:192], tph[:])                # A1
            nc.scalar.activation(spl[:, 0:64], spl[:, 128:192], COPY, scale=SC)  # A1s
            sptmp = gates.tile([128, 64], F32, name="sptmp")
            nc.vector.tensor_sub(sptmp[:], tph[:], spl[:, 128:192])
            nc.vector.tensor_scalar_mul(spl[:, 64:128], sptmp[:], SC)     # A2
            hsl = dram.tile([128, SLOT], F16, name="hsl")
            nc.sync.dma_start(hsl[:], spl[:])
            hgat = dram.tile([NC * 128, SLOT], F16, name="hgat", addr_space="Shared")
            nc.gpsimd.collective_compute(
                "AllGather",
                OP.bypass,
                replica_groups=RG,
                ins=[hsl[:].opt()],
                outs=[hgat[:].opt()],
            )
            # keep the PE busy through the AllGather wait so the tensor
            # engine doesn't drop out of its fast p-state before the dense
            wdum = dpsum.tile([128, 512], F32, name="dpr")
            for _ in range(18):
                nc.tensor.matmul(
                    wdum[:, 0:500], lhsT=hall0[:, 0:128], rhs=wd1[:, 0:500],
                    start=True, stop=True, skip_group_check=True,
                )

            # per-slot unpack: the first K-chunk matmuls only need slot 0
            hv = hpool.tile([128, NC * SLOT], F16, name="hv")
            for s in range(NC):
                nc.sync.dma_start(
                    hv[:, s * SLOT : (s + 1) * SLOT],
                    hgat[s * 128 : (s + 1) * 128, :],
                )

            # ---- dense: logits = (A1s@W1 + A1@W2 + A2@W1)/SC ----
            logits = lpool.tile([64, VS], F32, name="logits")
            if t < T - 1:
                lmax_all = ampool.tile([64, 8 * NTILES], F32, name="lmax_all")
                lidx_all = ampool.tile([64, 8 * NTILES], U32, name="lidx_all")
            for n in range(NTILES):
                c0n, ts = TOFF[n], TSZ[n]
                pr = dpsum.tile([128, 512], F32, name="dpr")
                for k in range(KD):
                    nc.tensor.matmul(
                        pr[:, 0:ts],
                        lhsT=hv[:, SLOT * k : SLOT * k + 128],
                        rhs=wd1[:, VS * k + c0n : VS * k + c0n + ts],
                        start=(k == 0),
                        stop=False,
                    )
                for k in range(KD):
                    nc.tensor.matmul(
                        pr[0:64, 0:ts],
                        lhsT=hv[:, SLOT * k + 128 : SLOT * (k + 1)],
                        rhs=wd2[:, VS * k + c0n : VS * k + c0n + ts],
                        start=False,
                        stop=(k == KD - 1),
                        skip_group_check=True,
                    )
                hsb = gates.tile([64, 512], F32, name="hsb")
                nc.scalar.activation(
                    hsb[:, 0:ts], pr[64:128, 0:ts], COPY, scale=1.0 / SC
                )
                lseg = logits[:, c0n : c0n + ts]
                if has_bd:
                    tmpl = gates.tile([64, 512], F32, name="tmpl")
                    nc.vector.scalar_tensor_tensor(
                        tmpl[:, 0:ts], pr[0:64, 0:ts], 1.0 / SC, hsb[:, 0:ts],
                        OP.mult, OP.add
                    )
                    nc.vector.tensor_add(
                        lseg, tmpl[:, 0:ts], bd[:, c0n : c0n + ts]
                    )
                else:
                    nc.vector.scalar_tensor_tensor(
                        lseg, pr[0:64, 0:ts], 1.0 / SC, hsb[:, 0:ts],
                        OP.mult, OP.add
                    )
                if t < T - 1:
                    nc.vector.max(
                        out=lmax_all[:, 8 * n : 8 * (n + 1)],
                        in_=lseg,
                    )
                    nc.vector.max_index(
                        lidx_all[:, 8 * n : 8 * (n + 1)],
                        lmax_all[:, 8 * n : 8 * (n + 1)],
                        lseg,
                    )
                # stream the output during the dense so the DMA engines are
                # quiet when the argmax AllGather's SDMA work needs them
                nc.sync.dma_start(out_d[t, :, c0n : c0n + ts], lseg)

            if t == T - 1:
                break

            # next step's h-part matmuls fill the PE during the argmax chain
            zps_cur = zpsum.tile([128, 512], F32, name="zps")
            emit_z_h(zps_cur, hv)
            # p-state warming through the argmax-AllGather idle window
            adum = dpsum.tile([128, 512], F32, name="dpr")
            for _ in range(14):
                nc.tensor.matmul(
                    adum[:, 0:500], lhsT=hall0[:, 0:128], rhs=wd1[:, 0:500],
                    start=True, stop=True, skip_group_check=True,
                )

            # ---- local top-1 (first-occurrence ties) ----
            v3d = lmax_all[:].rearrange("b (g j) -> b g j", j=8)
            i3d = lidx_all[:].rearrange("b (g j) -> b g j", j=8)
            vals8 = v3d[:, :, 0]
            pk = ampool.tile([64, 2], F32, name="pk")
            nc.vector.tensor_reduce(pk[:, 0:1], vals8, axis=X, op=OP.max)
            gidx8 = ampool.tile([64, NTILES], F32, name="gidx8")
            nc.vector.tensor_tensor(out=gidx8[:], in0=i3d[:, :, 0], in1=offs[:], op=OP.add)
            leq = ampool.tile([64, NTILES], U32, name="leq")
            nc.vector.tensor_tensor(
                out=leq[:], in0=vals8, in1=pk[:, 0:1].to_broadcast([64, NTILES]),
                op=OP.is_equal,
            )
            lpick = ampool.tile([64, NTILES], F32, name="lpick")
            nc.vector.memset(lpick[:], 1.0e9)
            nc.vector.copy_predicated(lpick[:], leq[:], gidx8[:])
            nc.vector.tensor_reduce(pk[:, 1:2], lpick[:], axis=X, op=OP.min)

            # ---- global argmax combine via tiny AllGather ----
            amin = dram.tile([64, 2], F32, name="amin")
            nc.sync.dma_start(amin[:], pk[:])
            amout = dram.tile([NC * 64, 2], F32, name="amout", addr_space="Shared")
            nc.gpsimd.collective_compute(
                "AllGather",
                OP.bypass,
                replica_groups=RG,
                ins=[amin[:].opt()],
                outs=[amout[:].opt()],
            )
            cand = ampool.tile([64, 16], F32, name="cand")
            nc.sync.dma_start(
                cand[:].rearrange("b (c j) -> b c j", j=2),
                amout[:].rearrange("(c b) j -> b c j", c=NC),
            )
            c3 = cand[:].rearrange("b (c j) -> b c j", j=2)
            vals = c3[:, :, 0]
            idxs = c3[:, :, 1]
            gmx = ampool.tile([64, 1], F32, name="gmx")
            nc.vector.tensor_reduce(gmx[:], vals, axis=X, op=OP.max)
            eq = ampool.tile([64, 8], U32, name="eq")
            nc.vector.tensor_tensor(
                out=eq[:], in0=vals, in1=gmx[:].to_broadcast([64, 8]), op=OP.is_equal
            )
            pick = ampool.tile([64, 8], F32, name="pick")
            nc.vector.memset(pick[:], 1.0e9)
            nc.vector.copy_predicated(pick[:], eq[:], idxs)
            gixf = ampool.tile([64, 1], F32, name="gixf")
            nc.vector.tensor_reduce(gixf[:], pick[:], axis=X, op=OP.min)
            gi32 = ampool.tile([64, 1], I32, name="gi32")
            nc.vector.tensor_copy(gi32[:], gixf[:])

            # ---- gather next step's x-side pre-activations ----
            zx_next = zxpool.tile([64, 512], F32, name="zx_sb")
            nc.gpsimd.indirect_dma_start(
                out=zx_next[:],
                out_offset=None,
                in_=videmb_d[:],
                in_offset=bass.IndirectOffsetOnAxis(ap=gi32[:, :1], axis=0),
            )
            zx_cur = zx_next

    nc.compile()
    return nc


def make_in_maps(inputs: dict, T: int = T_FULL):
    h0 = np.ascontiguousarray(np.asarray(inputs["h0"], np.float32))
    c0 = np.ascontiguousarray(np.asarray(inputs["c0"], np.float32))
    emb = np.ascontiguousarray(np.asarray(inputs["emb"], np.float32))
    W_ih = np.asarray(inputs["W_ih"], np.float32)
    W_hh = np.asarray(inputs["W_hh"], np.float32)
    b = np.asarray(inputs["b"], np.float32)
    W_d = np.asarray(inputs["W_dense"], np.float32)
    b_d = np.asarray(inputs["b_dense"], np.float32)

    has_bd = bool(np.any(b_d != 0))

    ident = np.eye(64, dtype=np.float32)

    emb64 = emb.astype(np.float64)
    Wih64 = W_ih.astype(np.float64)
    b64 = b.astype(np.float64)

    def split16(M):
        M1 = M.astype(np.float16)
        M2 = ((M - M1.astype(np.float32)) * SC).astype(np.float16)
        return M1, M2

    in_maps = []
    for c in range(NC):
        # gate-column order (i, f, o, g); g columns carry x2 so a single
        # tanh(0.5*z) activation serves sigmoid gates and the g tanh alike
        ucols = np.concatenate(
            [np.arange(g * U + 128 * c, g * U + 128 * (c + 1)) for g in (0, 1, 3, 2)]
        )
        gscale = np.ones(512, np.float64)
        gscale[384:] = 2.0
        videmb = ((emb64 @ Wih64[:, ucols] + b64[ucols]) * gscale).astype(np.float32)
        zx0 = np.ascontiguousarray(np.repeat(videmb[GO][None, :], B, axis=0))

        Whh_c = (W_hh[:, ucols] * gscale.astype(np.float32)).reshape(8, 128, 512)
        Whh1, Whh2 = split16(Whh_c)
        layhh = lambda M: np.ascontiguousarray(
            M.transpose(1, 0, 2).reshape(128, 8 * 512)
        )

        Wd_c = W_d[:, VS * c : VS * (c + 1)].reshape(8, 128, VS)
        W1, W2 = split16(Wd_c)
        lay16 = lambda M: np.ascontiguousarray(
            M.transpose(1, 0, 2).reshape(128, 8 * VS)
        )

        # pre-cell h0 splits, slot j = units of core j: [A1s | A2 | A1]
        h0T = h0.T.reshape(8, 128, 64)            # [8, 128, 64]
        A1 = h0T.astype(np.float16)
        A1s = (A1.astype(np.float32) * SC).astype(np.float16)
        A2 = ((h0T - A1.astype(np.float32)) * SC).astype(np.float16)
        hall0 = np.concatenate([A1s, A2, A1], axis=2)  # [8, 128, 192]
        hall0 = np.ascontiguousarray(
            hall0.transpose(1, 0, 2).reshape(128, NC * SLOT)
        )

        c0_c = np.ascontiguousarray(c0[:, 128 * c : 128 * (c + 1)])
        offs8 = np.repeat(
            (np.asarray(TOFF, dtype=np.float32) + VS * c)[None, :], B, axis=0
        )
        m = {
            "hall0": hall0,
            "c0": c0_c,
            "videmb": videmb,
            "zx0": zx0,
            "whh1": layhh(Whh1),
            "whh2": layhh(Whh2),
            "wd1": lay16(W1),
            "wd2": lay16(W2),
            "offs8": np.ascontiguousarray(offs8),
            "ident": ident,
        }
        if has_bd:
            m["bd"] = np.ascontiguousarray(
                np.repeat(b_d[VS * c : VS * (c + 1)][None, :], B, axis=0)
            )
        in_maps.append(m)
    return in_maps, has_bd, False


def assemble_output(results, T: int = T_FULL):
    parts = [np.asarray(r["out"]).reshape(T, B, VS) for r in results]
    full = np.concatenate(parts, axis=2)  # [T, 64, 32000]
    return np.ascontiguousarray(full.transpose(1, 0, 2))  # [64, T, 32000]


def kernel(**inputs) -> np.ndarray:
    in_maps, has_bd, _ = make_in_maps(inputs)
    nc = build_program(T_FULL, has_bd=has_bd)
    res = run_bass_kernel_spmd(nc, in_maps, core_ids=list(range(NC)))
    return assemble_output(res.results)


if __name__ == "__main__":
    print("kernel module OK")
